# revision 21
# baseline (speedup 1.0000x reference)
"""Trainium2 Bass kernel for nn_FIoUCriterion (pairwise-overlap IoU-style loss).

Strategy (8 NeuronCores, data-parallel over batch):
  - Host: cast masks (32,64,128,128) f32 -> bf16, reshape per-core shard to
    (256, 16384) rows=(4 local batches x 64 nodes); derive the static per-pair
    beta / weight matrices from `nodes` (pure index logic).
  - Device (per core): batched DMA-xbar-transpose loads pixel-major tiles,
    one fused DVE op y = max(x,-1)+1 (= 2*m), PE grams with 2-batch stacking:
    for each 128-pixel chunk and each batch pair, matmul(lhsT=tile, rhs=tile)
    accumulates y@y.T into PSUM, plus a N=1 matmul against a ones column
    accumulates s = sum_k y.  Epilogue: r = 1/s, C = gram*r_i,
    cr_contrib = max(C, C^T) (valid since gram>=0), accumulate over local
    pairs, AllGather+local-sum of the (64,64) partials across 8 cores, then
    loss = sum(|beta - cr_sum/64| * wgt2) with symmetrized normalized weights
    wgt2 = (wgt + wgt^T) / (2*sum(wgt)).
  - Scale bookkeeping: y = 2m  => gram_psum = 4*gram, s_psum = 2*s,
    max(C,C^T) = 2*cr_b; sum over 32 batches then *1/64 gives mean cr.
"""

import numpy as np
import ml_dtypes

N_CORES = 8
B, N, H, W = 32, 64, 128, 128
HW = H * W
B_LOC = B // N_CORES          # 4 batches per core
ROWS = B_LOC * N              # 256
N_PAIRS = B_LOC // 2          # 2 stacked pairs per core
N_CHUNK = HW // 128           # 128 pixel chunks
N_SEPARATE = 7
N_FLEXIBLE = 2

_cached = {}

# matmul layout: "plain" = per-chunk fp8 matmul (FWL-friendly);
# "dr" = interleaved DoubleRow fp8 (rejected by dual-fp8 LW ISA checks)
MM = "plain"


def _build_bass(with_collective: bool = True, bench_loop: int | None = None,
                phase: str = "full", variant: int = 1,
                fold_mode: str = "pe", cc_mode: str = "ag", dual_ring: bool = False,
                calls_by_pair=None, stream_bufs: int = 6):
    import contextlib
    import concourse.bacc as bacc
    import concourse.mybir as mybir
    import concourse.tile as tile

    f32 = mybir.dt.float32
    bf16 = mybir.dt.bfloat16
    Alu = mybir.AluOpType

    nc = bacc.Bacc("TRN2", target_bir_lowering=False, debug=False, num_devices=N_CORES)
    xb = nc.dram_tensor("xb", [ROWS, HW], bf16, kind="ExternalInput")
    beta_d = nc.dram_tensor("beta", [N, N], f32, kind="ExternalInput")
    wgt2_d = nc.dram_tensor("wgt2", [N, N], f32, kind="ExternalInput")
    loss_d = nc.dram_tensor("loss", [1, 1], f32, kind="ExternalOutput")

    def emit(tc, const, stream, ep, gpsum, tpsum, dram):
        # --- constants ---
        ones_bf = const.tile([128, 1], bf16)
        nc.vector.memset(ones_bf[:], 1.0)
        ones_f32 = const.tile([N, 1], f32)
        nc.vector.memset(ones_f32[:], 1.0)
        ident = const.tile([128, 128], f32)
        from concourse import masks as masks_lib
        masks_lib.make_identity(nc, ident[:])
        beta_t = const.tile([N, N], f32)
        nc.sync.dma_start(beta_t[:], beta_d[:])
        wgt2_t = const.tile([N, N], f32)
        nc.sync.dma_start(wgt2_t[:], wgt2_d[:])

        # --- PSUM accumulators (persist across the stream) ---
        g_acc = [gpsum.tile([128, 128], f32, name=f"g_acc{p}") for p in range(N_PAIRS)]
        s_acc = [gpsum.tile([128, 1], f32, name=f"s_acc{p}") for p in range(N_PAIRS)]

        bench_cm = (tc.For_i(0, bench_loop, 1, hint_engines=(mybir.EngineType.PE,))
                    if bench_loop else contextlib.nullcontext())
        bench_cm.__enter__()

        acc = ep.tile([128, 128], f32)

        def pair_epilogue(p):
            r = ep.tile([128, 1], f32, name=f"r{p}")
            nc.vector.reciprocal(r[:], s_acc[p][:])
            C = ep.tile([128, 128], f32, name=f"C{p}")
            nc.vector.tensor_scalar_mul(C[:], g_acc[p][:], r[:])
            CT = tpsum.tile([128, 128], f32, name=f"CT{p}", tag="CT", bufs=2)
            nc.tensor.transpose(CT[:], C[:], ident[:])
            if p == 0:
                nc.vector.tensor_max(acc[:], C[:], CT[:])
            else:
                mx = ep.tile([128, 128], f32, name=f"mx{p}")
                nc.vector.tensor_max(mx[:], C[:], CT[:])
                nc.vector.tensor_add(acc[:], acc[:], mx[:])

        if phase == "dmaplain":
            for half in range(2):
                for q in range(4):
                    tp = stream.tile([128, 4096], bf16, name="tp")
                    nc.sync.dma_start(tp[:], xb[half * 128:(half + 1) * 128,
                                                q * 4096:(q + 1) * 4096])
        elif variant == 1:
            # --- streaming: batched xbar transposes, X chunks per DMA ---
            X = 16                       # 128-pixel chunks per transpose call
            n_big = N_CHUNK // X         # 8 mega-tiles
            for c2 in range(n_big if phase != "noop" else 0):
                t = stream.tile([128, X * ROWS], bf16, name="t")
                c0 = c2 * X * 128
                # out[p, i, f] = xb[f, c0 + i*128 + p]  (verified on HW)
                nc.sync.dma_start(t[:].rearrange("p (x f) -> p x f", x=X),
                                  xb[0:ROWS, c0:c0 + X * 128], transpose=True)
                # y = max(x, -1) + 1   (= 2*m), in place, bf16 4x mode
                if phase != "dma":
                    nc.vector.tensor_scalar(t[:], t[:], -1.0, 1.0, Alu.max, Alu.add)
                if phase in ("dma", "dma_relu"):
                    continue
                for k in range(X):       # chunk within the mega tile
                    first = (c2 == 0 and k == 0)
                    last = (c2 == n_big - 1 and k == X - 1)
                    for p in range(N_PAIRS):
                        sl = t[:, k * ROWS + p * 128: k * ROWS + (p + 1) * 128]
                        nc.tensor.matmul(g_acc[p][:], lhsT=sl, rhs=sl,
                                         start=first, stop=last)
                        nc.tensor.matmul(s_acc[p][:], lhsT=sl, rhs=ones_bf[:],
                                         start=first, stop=last)
        else:
            # --- v2: pair-split streaming; pair0's epilogue overlaps pair1's
            # stream.  Tapered call sizes shrink the serial tail. ---
            CALLS_BY_PAIR = calls_by_pair or [
                [8, 16, 40, 64],          # pair0: fill the pipeline fast
                [48, 40, 24, 8, 8],       # pair1: shrink the serial tail
            ]
            dma_i = 0
            for p in range(N_PAIRS if phase != "noop" else 0):
                CALLS = CALLS_BY_PAIR[p]
                assert sum(CALLS) == N_CHUNK
                row0 = p * 128
                c0 = 0
                for ci, Xc in enumerate(CALLS):
                    # dedicated buffer per call (whole shard = 64KB/partition):
                    # no slot-reuse WAW stalls, DMA queue can run arbitrarily deep
                    t = stream.tile([128, Xc * 128], bf16, name="t",
                                    tag=f"t{p}_{ci}", bufs=1)
                    tv = t[:, 0:Xc * 128]
                    # alternate the two HWDGE rings (SP / ACT)
                    eng = nc.sync if (dma_i % 2 == 0 or not dual_ring) else nc.scalar
                    dma_i += 1
                    eng.dma_start(
                        tv.rearrange("q (x f) -> q x f", x=Xc),
                        xb[row0:row0 + 128, c0 * 128:(c0 + Xc) * 128],
                        transpose=True)
                    if phase != "dma":
                        # split relu so the first chunks' matmuls can start
                        # while the rest of the call is still in the DVE
                        h = (Xc // 2) * 128 if Xc > 8 else Xc * 128
                        nc.vector.tensor_scalar(t[:, 0:h], t[:, 0:h],
                                                -1.0, 1.0, Alu.max, Alu.add)
                        if h < Xc * 128:
                            nc.vector.tensor_scalar(t[:, h:Xc * 128], t[:, h:Xc * 128],
                                                    -1.0, 1.0, Alu.max, Alu.add)
                    if phase not in ("dma", "dma_relu"):
                        for k in range(Xc):
                            first = (ci == 0 and k == 0)
                            last = (ci == len(CALLS) - 1 and k == Xc - 1)
                            sl = t[:, k * 128:(k + 1) * 128]
                            nc.tensor.matmul(g_acc[p][:], lhsT=sl, rhs=sl,
                                             start=first, stop=last)
                            nc.tensor.matmul(s_acc[p][:], lhsT=sl, rhs=ones_bf[:],
                                             start=first, stop=last)
                    c0 += Xc
                if phase == "full":
                    pair_epilogue(p)

        if phase in ("noop", "dma", "dmaplain", "dma_relu", "stream"):
            lout0 = ep.tile([1, 1], f32)
            nc.vector.memset(lout0[:], 0.0)
            nc.sync.dma_start(loss_d[:], lout0[:])
            bench_cm.__exit__(None, None, None)
            return

        if variant == 1:
            for p in range(N_PAIRS):
                pair_epilogue(p)

        # fold the two stacked 64-blocks: local cr partial (64,64).
        if variant == 1 or fold_mode == "dma":
            # engines can't move data across partitions; small SBUF->SBUF DMA
            blk1 = ep.tile([N, N], f32)
            nc.sync.dma_start(blk1[:], acc[N:128, N:128])
            crl = ep.tile([N, N], f32)
            nc.vector.tensor_add(crl[:], acc[0:N, 0:N], blk1[:])
        else:
            # PE transpose moves block1 down to partitions 0:64; the block is
            # symmetric so the transpose is a no-op on values.
            blk1p = tpsum.tile([N, N], f32, name="blk1p")
            nc.tensor.transpose(blk1p[:], acc[N:128, N:128], ident[N:128, N:128])
            crl = ep.tile([N, N], f32)
            nc.vector.tensor_add(crl[:], acc[0:N, 0:N], blk1p[:])

        # --- combine partials across the 8 cores ---
        # AllGather (floor ~4.6us on 8 cores) + local sum beats AllReduce
        # (floor ~9.7us) at this size.
        if with_collective and cc_mode == "ar":
            cc_in0 = dram.tile([N, N], f32)
            cc_out0 = dram.tile([N, N], f32, addr_space="Shared")
            nc.sync.dma_start(cc_in0[:], crl[:])
            nc.gpsimd.collective_compute(
                "AllReduce", Alu.add,
                replica_groups=[list(range(N_CORES))],
                ins=[cc_in0.opt()], outs=[cc_out0.opt()],
            )
            crs = ep.tile([N, N], f32, name="crs_ar")
            nc.sync.dma_start(crs[:], cc_out0[:])
        elif with_collective:
            cc_in = dram.tile([N, N], f32)
            cc_ag = dram.tile([N_CORES * N, N], f32, addr_space="Shared")
            nc.sync.dma_start(cc_in[:], crl[:])
            nc.gpsimd.collective_compute(
                "AllGather", Alu.bypass,
                replica_groups=[list(range(N_CORES))],
                ins=[cc_in.opt()], outs=[cc_ag.opt()],
            )
            # gather back as (64, r, 64): S[i, r, j] = AG[r*64+i, j]
            sg = ep.tile([N, N_CORES * N], f32)
            nc.sync.dma_start(
                sg[:].rearrange("i (r j) -> i r j", r=N_CORES),
                cc_ag[:].rearrange("(r i) j -> i r j", r=N_CORES))
            crs = ep.tile([N, N], f32)
            # reduce over r: view free dim as (j outer, r inner) and reduce X
            nc.vector.tensor_reduce(
                crs[:], sg[:].rearrange("i (r j) -> i j r", r=N_CORES),
                mybir.AxisListType.X, Alu.add)
        else:
            crs = crl

        # --- final reduction ---
        u = ep.tile([N, N], f32)
        # u = (crs * 1/64) - beta
        nc.vector.scalar_tensor_tensor(u[:], crs[:], 1.0 / 64.0, beta_t[:],
                                       Alu.mult, Alu.subtract)
        v = ep.tile([N, N], f32)
        nc.vector.tensor_mul(v[:], u[:], wgt2_t[:])
        vr = ep.tile([N, 1], f32)
        nc.vector.tensor_reduce(vr[:], v[:], mybir.AxisListType.X, Alu.add,
                                apply_absolute_value=True)
        lps = tpsum.tile([1, 1], f32)
        nc.tensor.matmul(lps[:], lhsT=vr[:], rhs=ones_f32[:], start=True, stop=True)
        lout = ep.tile([1, 1], f32)
        nc.vector.tensor_copy(lout[:], lps[:])
        nc.sync.dma_start(loss_d[:], lout[:])

        bench_cm.__exit__(None, None, None)

    with tile.TileContext(nc) as tc:
        with tc.tile_pool(name="const", bufs=1) as const, \
             tc.tile_pool(name="stream", bufs=stream_bufs) as stream, \
             tc.tile_pool(name="ep", bufs=1) as ep, \
             tc.tile_pool(name="gpsum", bufs=1, space="PSUM") as gpsum, \
             tc.tile_pool(name="tpsum", bufs=1, space="PSUM") as tpsum, \
             tc.tile_pool(name="dram", bufs=1, space="DRAM") as dram:
            emit(tc, const, stream, ep, gpsum, tpsum, dram)

    nc.compile()
    return nc


def _build_bass3(with_collective: bool = True, bench_loop: int | None = None,
                 phase: str = "full", calls_by_pair=None, dve_frac: float = 0.62,
                 act_frac: float = 0.38, use_pool: bool = False,
                 combine: str = "ag"):
    """Variant 3.1: fp8e4 end-to-end, fused gram+rowsum matmul.

    Host ships x (pre-relu) quantized to fp8e4 in a pair-major pixel-partition
    layout with an extra zero column per 128-pixel chunk:
      xb[p, pr*16512 + x*129 + j] = x[pr*128+j, x*128+p]  (j<128), 0 for j=128.
    All DMAs are plain contiguous loads (no xbar transpose, ~4MB/core).
    Device: y = relu(x+1) (= 2*m) turns the zero column into exactly 1.0 and is
    split across DVE / ACT / GPSIMD so all three elementwise engines chew
    concurrently.  One DoubleRow fp8 matmul per double-chunk then contracts TWO
    chunks AND the row-sums in a single instruction:
      gs_acc[p][:, 0:128] += sl0.T@sl0 + sl1.T@sl1   (gram)
      gs_acc[p][:, 128]   += sl0.T@1  + sl1.T@1      (s)
    Cross-core combine (combine="rdma"): 8 single-dest remote_dma_broadcast
    calls with XOR-relative dests implement a SPMD-symmetric all-gather of the
    (64,64) crl partial (padded to 128 partitions) over the intra-device DMA
    fabric; each receiver slot-indexes by XOR so no core-id arithmetic is
    needed.  A DVE-bound monotonic semaphore (+2 per arrival x 8 = 16/iter)
    orders the local 8-way reduce.  ~2us vs ~9.4us for collective AllGather +
    bounce DMAs, and it can sit inside the timed For_i loop.
    """
    import contextlib
    import concourse.bacc as bacc
    import concourse.mybir as mybir
    import concourse.tile as tile

    f32 = mybir.dt.float32
    f8 = mybir.dt.float8e4
    Alu = mybir.AluOpType
    Act = mybir.ActivationFunctionType
    DR = mybir.MatmulPerfMode.DoubleRow

    PAIR_COLS = N_CHUNK * 129          # 16512 columns per pair

    nc = bacc.Bacc("TRN2", target_bir_lowering=False, debug=False, num_devices=N_CORES)
    xb = nc.dram_tensor("xb", [128, 2 * PAIR_COLS], f8, kind="ExternalInput")
    beta_d = nc.dram_tensor("beta", [N, N], f32, kind="ExternalInput")
    wgt2_d = nc.dram_tensor("wgt2", [N, N], f32, kind="ExternalInput")
    loss_d = nc.dram_tensor("loss", [1, 1], f32, kind="ExternalOutput")

    def emit(tc, const, stream, ep, gpsum, tpsum, dram):
        # --- constants ---
        ones_f32 = const.tile([N, 1], f32)
        nc.vector.memset(ones_f32[:], 1.0)
        ident = const.tile([128, 128], f32)
        from concourse import masks as masks_lib
        masks_lib.make_identity(nc, ident[:])
        beta_t = const.tile([N, N], f32)
        nc.sync.dma_start(beta_t[:], beta_d[:])
        wgt2_t = const.tile([N, N], f32)
        nc.sync.dma_start(wgt2_t[:], wgt2_d[:])

        if combine == "rdma":
            from concourse.bass import MonotonicSemaphore
            R8 = const.tile([128, N_CORES * N], f32)   # gather slots
            nc.vector.memset(R8[:], 0.0)
            V = const.tile([128, N], f32)              # crl padded to 128 parts
            nc.vector.memset(V[:], 0.0)
            A_h = nc.monotonic_semaphore(0).sem()
            ms_dve = MonotonicSemaphore(nc.vector, A_h)
            L_h = nc.alloc_semaphore("rdma_local")

        gs_acc = [gpsum.tile([128, 129], f32, name=f"gs_acc{p}") for p in range(2)]

        bench_cm = (tc.For_i(0, bench_loop, 1, hint_engines=(mybir.EngineType.PE,))
                    if bench_loop else contextlib.nullcontext())
        bench_cm.__enter__()

        acc = ep.tile([128, 128], f32)

        def pair_epilogue(p):
            r = ep.tile([128, 1], f32, name=f"r{p}")
            nc.vector.reciprocal(r[:], gs_acc[p][:, 128:129])
            C = ep.tile([128, 128], f32, name=f"C{p}")
            nc.vector.tensor_scalar_mul(C[:], gs_acc[p][:, 0:128], r[:])
            CT = tpsum.tile([128, 128], f32, name=f"CT{p}", tag="CT", bufs=2)
            nc.tensor.transpose(CT[:], C[:], ident[:])
            if p == 0:
                nc.vector.tensor_max(acc[:], C[:], CT[:])
            else:
                mx = ep.tile([128, 128], f32, name=f"mx{p}")
                nc.vector.tensor_max(mx[:], C[:], CT[:])
                nc.vector.tensor_add(acc[:], acc[:], mx[:])

        CALLS_BY_PAIR = calls_by_pair or [
            [16, 48, 64],            # pair0: fill the pipeline fast
            [64, 48, 16],            # pair1: shrink the serial tail
        ]
        for p in range(2 if phase != "noop" else 0):
            CALLS = CALLS_BY_PAIR[p]
            assert sum(CALLS) == N_CHUNK and all(x % 2 == 0 for x in CALLS)
            base = p * PAIR_COLS
            c0 = 0
            for ci, Xc in enumerate(CALLS):
                cols = Xc * 129
                t = stream.tile([128, cols], f8, name="t",
                                tag=f"t{p}_{ci}", bufs=1)
                nc.sync.dma_start(t[:], xb[:, base + c0 * 129:base + c0 * 129 + cols])
                if phase not in ("dma", "pe"):
                    # y = relu(x + 1) (= 2*m), in place; split across the
                    # elementwise engines (DVE / ACT / GPSIMD) by double-chunk
                    n_dc = Xc // 2
                    b1 = int(round(dve_frac * n_dc))
                    b2 = int(round((dve_frac + act_frac) * n_dc)) if use_pool else n_dc
                    b1, b2 = max(b1, 1), max(min(b2, n_dc), 1)
                    h1, h2 = b1 * 258, b2 * 258
                    nc.vector.tensor_scalar(t[:, 0:h1], t[:, 0:h1],
                                            -1.0, 1.0, Alu.max, Alu.add)
                    if h1 < h2:
                        nc.scalar.activation(t[:, h1:h2], t[:, h1:h2],
                                             Act.Relu, bias=1.0, scale=1.0)
                    if h2 < cols:
                        nc.gpsimd.tensor_scalar(t[:, h2:cols], t[:, h2:cols],
                                                -1.0, 1.0, Alu.max, Alu.add)
                if phase not in ("dma", "dma_relu"):
                    if MM == "dr":
                        # interleaved double-chunk layout: col = dc*258 + j*2 + i
                        vv = t[:].rearrange("q (d j i) -> q d i j", j=129, i=2)
                        for kk in range(Xc // 2):
                            first = (ci == 0 and kk == 0)
                            last = (ci == len(CALLS) - 1 and kk == Xc // 2 - 1)
                            nc.tensor.matmul(gs_acc[p][:],
                                             lhsT=vv[:, kk, :, 0:128],
                                             rhs=vv[:, kk, :, :],
                                             start=first, stop=last, perf_mode=DR)
                    else:
                        v = t[:].rearrange("q (x j) -> q x j", j=129)
                        for k in range(Xc):
                            first = (ci == 0 and k == 0)
                            last = (ci == len(CALLS) - 1 and k == Xc - 1)
                            nc.tensor.matmul(gs_acc[p][:],
                                             lhsT=v[:, k, 0:128],
                                             rhs=v[:, k, :],
                                             start=first, stop=last)
                c0 += Xc
            if phase == "full":
                pair_epilogue(p)

        if phase in ("noop", "dma", "dma_relu", "stream", "pe"):
            lout0 = ep.tile([1, 1], f32)
            nc.vector.memset(lout0[:], 0.0)
            nc.sync.dma_start(loss_d[:], lout0[:])
            bench_cm.__exit__(None, None, None)
            return

        # fold the two stacked 64-blocks: PE transpose moves block1 down to
        # partitions 0:64 (block is symmetric so transpose is a value no-op).
        blk1p = tpsum.tile([N, N], f32, name="blk1p")
        nc.tensor.transpose(blk1p[:], acc[N:128, N:128], ident[N:128, N:128])

        if combine == "rdma":
            # fold straight into the broadcast source tile (parts 64: junk)
            nc.vector.tensor_add(V[0:N, :], acc[0:N, 0:N], blk1p[:])
            # all-gather: call d lands my V in slot d of core (me XOR d);
            # receiver slot d therefore holds core (me XOR d)'s partial.
            for d in range(N_CORES):
                rd = [None] * N_CORES
                rd[d] = (0, d)
                nc.gpsimd.remote_dma_broadcast(
                    R8[:, d * N:(d + 1) * N], V[:],
                    remote_sem=A_h, local_sem=L_h, rdests=rd)
            nc.gpsimd.trigger_dma(count=None)
            ms_dve.wait_inc(2 * N_CORES)    # +2 per arriving slot-write
            crs = ep.tile([N, N], f32)
            nc.vector.tensor_reduce(
                crs[:], R8[0:N, :].rearrange("i (r j) -> i j r", r=N_CORES),
                mybir.AxisListType.X, Alu.add)
        elif combine == "ag" and with_collective:
            crl = ep.tile([N, N], f32)
            nc.vector.tensor_add(crl[:], acc[0:N, 0:N], blk1p[:])
            cc_in = dram.tile([N, N], f32)
            cc_ag = dram.tile([N_CORES * N, N], f32, addr_space="Shared")
            nc.sync.dma_start(cc_in[:], crl[:])
            nc.gpsimd.collective_compute(
                "AllGather", Alu.bypass,
                replica_groups=[list(range(N_CORES))],
                ins=[cc_in.opt()], outs=[cc_ag.opt()],
            )
            sg = ep.tile([N, N_CORES * N], f32)
            nc.sync.dma_start(
                sg[:].rearrange("i (r j) -> i r j", r=N_CORES),
                cc_ag[:].rearrange("(r i) j -> i r j", r=N_CORES))
            crs = ep.tile([N, N], f32)
            nc.vector.tensor_reduce(
                crs[:], sg[:].rearrange("i (r j) -> i j r", r=N_CORES),
                mybir.AxisListType.X, Alu.add)
        else:
            crs = ep.tile([N, N], f32)
            nc.vector.tensor_add(crs[:], acc[0:N, 0:N], blk1p[:])

        u = ep.tile([N, N], f32)
        nc.vector.scalar_tensor_tensor(u[:], crs[:], 1.0 / 64.0, beta_t[:],
                                       Alu.mult, Alu.subtract)
        v = ep.tile([N, N], f32)
        nc.vector.tensor_mul(v[:], u[:], wgt2_t[:])
        vr = ep.tile([N, 1], f32)
        nc.vector.tensor_reduce(vr[:], v[:], mybir.AxisListType.X, Alu.add,
                                apply_absolute_value=True)
        lps = tpsum.tile([1, 1], f32)
        nc.tensor.matmul(lps[:], lhsT=vr[:], rhs=ones_f32[:], start=True, stop=True)
        lout = ep.tile([1, 1], f32)
        nc.vector.tensor_copy(lout[:], lps[:])
        nc.sync.dma_start(loss_d[:], lout[:])

        bench_cm.__exit__(None, None, None)

    with tile.TileContext(nc) as tc:
        with tc.tile_pool(name="const", bufs=1) as const, \
             tc.tile_pool(name="stream", bufs=1) as stream, \
             tc.tile_pool(name="ep", bufs=1) as ep, \
             tc.tile_pool(name="gpsum", bufs=1, space="PSUM") as gpsum, \
             tc.tile_pool(name="tpsum", bufs=1, space="PSUM") as tpsum, \
             tc.tile_pool(name="dram", bufs=1, space="DRAM") as dram:
            emit(tc, const, stream, ep, gpsum, tpsum, dram)

    nc.compile()
    return nc


def _host_prep3(masks: np.ndarray, nodes: np.ndarray):
    xb = masks.reshape(B, N, HW)
    shards = []
    for c in range(N_CORES):
        sh = xb[c * B_LOC:(c + 1) * B_LOC].reshape(ROWS, HW)
        q = sh.astype(ml_dtypes.float8_e4m3)
        # [pr, j, x, p] -> [p, pr, x, j], then append a zero column per chunk
        # (relu(0+1) = 1.0 exactly -> fused row-sum column in the matmul)
        t4 = q.reshape(2, 128, 128, 128).transpose(3, 0, 2, 1)
        a = np.zeros((128, 2, 128, 129), dtype=ml_dtypes.float8_e4m3)
        a[:, :, :, 0:128] = t4
        if MM == "dr":
            # interleave chunk pairs column-wise: col = dc*258 + j*2 + i
            a = a.reshape(128, 2, 64, 2, 129).transpose(0, 1, 2, 4, 3)
        shards.append(np.ascontiguousarray(a.reshape(128, 2 * 128 * 129)))

    t = np.where(nodes < N_SEPARATE, 0, np.where(nodes < N_SEPARATE + N_FLEXIBLE, 1, 2))
    ti, tj = t[:, None], t[None, :]
    has_f = (ti == 1) | (tj == 1)
    has_a = (ti == 2) | (tj == 2)
    include = ~(has_f & ~has_a)
    beta = ((ti == 2) ^ (tj == 2)).astype(np.float32)
    triu = np.triu(np.ones((N, N), bool), k=1)
    wgt = (include & triu).astype(np.float64)
    wgt2 = ((wgt + wgt.T) / (2.0 * wgt.sum())).astype(np.float32)
    return shards, beta, wgt2


def _host_prep(masks: np.ndarray, nodes: np.ndarray):
    xb = masks.reshape(B, N, HW).astype(ml_dtypes.bfloat16)
    shards = [np.ascontiguousarray(xb[c * B_LOC:(c + 1) * B_LOC].reshape(ROWS, HW))
              for c in range(N_CORES)]

    t = np.where(nodes < N_SEPARATE, 0, np.where(nodes < N_SEPARATE + N_FLEXIBLE, 1, 2))
    ti, tj = t[:, None], t[None, :]
    has_f = (ti == 1) | (tj == 1)
    has_a = (ti == 2) | (tj == 2)
    include = ~(has_f & ~has_a)
    beta = ((ti == 2) ^ (tj == 2)).astype(np.float32)
    triu = np.triu(np.ones((N, N), bool), k=1)
    wgt = (include & triu).astype(np.float64)
    wgt2 = ((wgt + wgt.T) / (2.0 * wgt.sum())).astype(np.float32)
    return shards, beta, wgt2


def kernel(masks: np.ndarray, nodes: np.ndarray) -> np.ndarray:
    from concourse.bass_utils import run_bass_kernel_spmd

    masks = np.asarray(masks, dtype=np.float32)
    nodes = np.asarray(nodes)
    shards, beta, wgt2 = _host_prep3(masks, nodes)

    if "nc" not in _cached:
        _cached["nc"] = _build_bass3()
    nc = _cached["nc"]

    in_maps = [{"xb": shards[c], "beta": beta, "wgt2": wgt2} for c in range(N_CORES)]
    try:
        res = run_bass_kernel_spmd(nc, in_maps, core_ids=list(range(N_CORES)))
    except Exception:
        res = run_bass_kernel_spmd(nc, in_maps, core_ids=list(range(N_CORES)))
    loss = np.float32(res.results[0]["loss"][0, 0])
    return np.asarray(loss, dtype=np.float32).reshape(())

